# revision 2
# baseline (speedup 1.0000x reference)
"""ColorCurveLearningLoss on 8 Trainium2 NeuronCores — v6.

Same 4(hi) x 8(lo) PE-matmul histogram as v2, two changes:

  1. rhs planes are STAIRCASE-VALUED, not masks: r~_j = bf16-RNE(
     idxb*0.125 + B_j) = 129 + floor((idx-j+8)/8), one chained mult+add
     TENSOR_SCALAR per plane straight from idxb (4x DVE mode).  This
     removes the uint16 AND op of v2.  Since r_j(idx) = hi(idx) +
     [lo(idx) >= j], the matmul rows are lo-cumulative sums plus a
     hi-contamination term H[slot] = sum(L_slot * hi), which the decode
     recovers LINEARLY from the ones-column totals: the count-slot H's
     from the hi-counts, the d-slot H's by solving the 4x4 sign system
     for the per-hi d-marginals D_h.  (All per-PSUM-cell partial sums
     stay < 2^24 so the count side stays integer-exact.)
  2. The one-time ones-plane presets run on GpSimd, off the DVE.
  3. The three PSUM->SBUF copies are deferred to after the last channel:
     ScalarE is strict FIFO, so a copy at a channel boundary used to
     block the next channel's ACTs until every matmul of the previous
     channel retired.
  4. The d = p-t subtract is emitted FIRST on the DVE (it does not need
     idxb), filling the wait for ScalarE at each tile start.
  5. Channel 0 ramps 512/1536/2048 columns (first matmul ~7us in) and
     channel 2 tapers 2048/1024/512/512 (short PE tail); all size
     classes share the same two full-size SBUF buffers via slicing.
     The very first tile's three input DMAs issue from three different
     engine queues so the transfers overlap.

Engine budget per [128,2048] tile: ScalarE 4 ACTs (idxb + 3 sign planes),
DVE 7 staircase TS + 4 TENSOR_TENSOR (d = p-t, 3x d*s_a), PE 128 matmuls.
p, t are pre-cast to bf16 on host (12.6 MB/core HBM).
"""

import numpy as np
import ml_dtypes

NB = 32
B, C, H, W = 16, 3, 512, 512
N_CORES = 8
B_PER_CORE = B // N_CORES
ELEMS_PER_CH = B_PER_CORE * H * W  # 524288
P = 128
FCH = ELEMS_PER_CH // P  # 4096 columns per channel
GROUP = 16
NGMAX = 2048 // GROUP
TILES = {0: [512, 1536, 2048], 1: [2048, 2048], 2: [2048, 1024, 512, 512]}
MAGIC = 128.5  # idxb = 129 + idx
RBASE = 129.0  # staircase planes hold RBASE + r_j

_CACHE = {}


def _build():
    import concourse.bass as bass
    import concourse.tile as tile
    from concourse import bacc, mybir

    nc = bacc.Bacc("TRN2", target_bir_lowering=False, debug=False,
                   num_devices=N_CORES)
    f32 = mybir.dt.float32
    bf16 = mybir.dt.bfloat16
    Identity = mybir.ActivationFunctionType.Identity
    Sign = mybir.ActivationFunctionType.Sign
    Alu = mybir.AluOpType

    sign_bias = [-(MAGIC + 8.0 * a) for a in (1, 2, 3)]
    for val in [MAGIC] + sign_bias:
        t = nc.alloc_sbuf_tensor(f"constx-{val}", [128, 1], f32)
        nc.gpsimd.memset(t.ap(), val)
        nc.const_aps.aps[(f32, val)] = t.ap()
    nc.all_engine_barrier()

    stair_bias = {j: 113.4375 - j / 8.0 for j in range(1, 8)}
    # z_j = idxb/8 + B_j lands in [128.69, 133.32] where bf16 spacing is
    # exactly 1.0; RNE(z_j) = RBASE + floor((idx-j+8)/8) with no ties.

    xin = nc.dram_tensor("xin", [C, P, FCH // GROUP, GROUP], f32,
                         kind="ExternalInput")
    pin = nc.dram_tensor("pin", [C, P, FCH // GROUP, GROUP], bf16,
                         kind="ExternalInput")
    tin = nc.dram_tensor("tin", [C, P, FCH // GROUP, GROUP], bf16,
                         kind="ExternalInput")
    out = nc.dram_tensor("out", [P, C * 128], f32,
                         kind="ExternalOutput")

    with tile.TileContext(nc) as tc:
        with (
            tc.tile_pool(name="inp", bufs=2) as inp,
            tc.tile_pool(name="work", bufs=2) as work,
            tc.tile_pool(name="acc", bufs=1) as accp,
            tc.tile_pool(name="ps", bufs=1, space="PSUM") as ps,
        ):
            psum = ps.tile([P, C, 512], f32, tag="psum", name="psum")
            res = accp.tile([P, C * 128], f32, name="res")

            it = 0
            for c in range(C):
                n_mm = sum(f // GROUP for f in TILES[c])
                mm_i = 0
                off = 0
                for F in TILES[c]:
                    NG = F // GROUP
                    g0 = off // GROUP
                    xt_b = inp.tile([P, NGMAX, GROUP], f32, tag="x",
                                    name="xt")
                    xt = xt_b[:, :NG, :]
                    pt_b = inp.tile([P, NGMAX, GROUP], bf16, tag="p",
                                    name="pt")
                    pt = pt_b[:, :NG, :]
                    tt_b = inp.tile([P, NGMAX, GROUP], bf16, tag="t",
                                    name="tt")
                    tt_in = tt_b[:, :NG, :]
                    if it == 0:
                        # spread the first tile's loads over three queues
                        nc.sync.dma_start(out=xt,
                                          in_=xin[c, :, g0:g0 + NG, :])
                        nc.scalar.dma_start(out=pt,
                                            in_=pin[c, :, g0:g0 + NG, :])
                        nc.gpsimd.dma_start(out=tt_in,
                                            in_=tin[c, :, g0:g0 + NG, :])
                    else:
                        nc.sync.dma_start(out=xt,
                                          in_=xin[c, :, g0:g0 + NG, :])
                        nc.sync.dma_start(out=pt,
                                          in_=pin[c, :, g0:g0 + NG, :])
                        nc.sync.dma_start(out=tt_in,
                                          in_=tin[c, :, g0:g0 + NG, :])

                    # idxb = 129 + floor(32x) via single ACT (bf16 RNE magic)
                    ix_b = work.tile([P, NGMAX, GROUP], bf16, tag="idxb",
                                     name="idxb")
                    idxb = ix_b[:, :NG, :]
                    nc.scalar.activation(out=idxb, in_=xt,
                                         func=Identity, bias=MAGIC,
                                         scale=float(NB))

                    # rhs: u[:, 0] = ones (preset, gpsimd); u[:, j] =
                    # 129 + floor((idx-j+8)/8)  (staircase via bf16 RNE)
                    u = work.tile([P, 8, NGMAX, GROUP], bf16, tag="u",
                                  name="u")
                    L = work.tile([P, NGMAX, 8 * GROUP], bf16, tag="L",
                                  name="L")
                    if it < 2:
                        nc.gpsimd.memset(u[:, 0, :, :], 1.0)
                        nc.gpsimd.memset(L[:, :, 4 * GROUP:5 * GROUP], 1.0)
                    # d = p - t first: needs only the DMAs, not idxb
                    nc.vector.tensor_tensor(
                        out=L[:, :NG, 0:GROUP], in0=pt, in1=tt_in,
                        op=Alu.subtract)
                    for j in range(1, 8):
                        nc.vector.tensor_scalar(
                            out=u[:, j, :NG, :], in0=idxb,
                            scalar1=0.125, scalar2=stair_bias[j],
                            op0=Alu.mult, op1=Alu.add)

                    def pl(a):
                        return L[:, :NG, a * GROUP:(a + 1) * GROUP]

                    # sign planes on ScalarE: s_a = sign(idx - 8a + 0.5)
                    for a in (1, 2, 3):
                        nc.scalar.activation(out=pl(4 + a), in_=idxb,
                                             func=Sign, bias=sign_bias[a - 1])
                    for a in (1, 2, 3):
                        nc.vector.tensor_tensor(
                            out=pl(a), in0=pl(0), in1=pl(4 + a),
                            op=Alu.mult)

                    for gi in range(NG):
                        nc.tensor.matmul(
                            psum[:, c, 0:128],
                            lhsT=L[:, gi, :],
                            rhs=u[:, :, gi, :],
                            start=(mm_i == 0),
                            stop=(mm_i == n_mm - 1),
                        )
                        mm_i += 1
                    off += F
                    it += 1

            for c in range(C):
                nc.scalar.copy(out=res[:, c * 128:(c + 1) * 128],
                               in_=psum[:, c, 0:128])
            nc.sync.dma_start(out=out[:], in_=res[:])

    nc.compile()
    return nc


def _get_nc():
    if "nc" not in _CACHE:
        _CACHE["nc"] = _build()
    return _CACHE["nc"]


def _shard(arr, core, bf=False):
    a = arr[core * B_PER_CORE:(core + 1) * B_PER_CORE]
    a = np.ascontiguousarray(np.transpose(a, (1, 0, 2, 3)))
    a = a.reshape(C, P, FCH // GROUP, GROUP)
    if bf:
        return a.astype(ml_dtypes.bfloat16)
    return a.astype(np.float32, copy=False)


def _make_in_maps(pred, target, input_img):
    pred = np.asarray(pred)
    target = np.asarray(target)
    input_img = np.asarray(input_img)
    in_maps = []
    for core in range(N_CORES):
        in_maps.append({
            "xin": _shard(input_img, core),
            "pin": _shard(pred, core, bf=True),
            "tin": _shard(target, core, bf=True),
        })
    return in_maps


def _decode(raw):
    """raw [P, C*128] -> per-(channel,bin) sums S and counts Cnt.

    V[slot, col] = sum_e L_slot(e) * R_col(e), R_0 = ones,
    R_j = RBASE + hi + [lo >= j] for j=1..7.
    Slots: [d, d*s1, d*s2, d*s3, ones, s1, s2, s3], s_a in {-1,+1}.
    """
    S = np.zeros((C, NB), np.float64)
    Cnt = np.zeros((C, NB), np.float64)
    for c in range(C):
        Pm = raw[:, c * 128:(c + 1) * 128].astype(np.float64)
        R = Pm.reshape(8, GROUP, 8, GROUP)
        V = np.einsum('aibi->ab', R)  # [slot, col]
        T = V[:, 0].copy()
        M = V[:, 1:] - RBASE * T[:, None]  # sum L*(hi + [lo>=j])

        # hi-marginal counts from sign totals
        N_tot = T[4]
        N_ge = np.zeros(5)
        N_ge[0] = N_tot
        for a in (1, 2, 3):
            N_ge[a] = (T[4 + a] + N_tot) / 2
        Nc = N_ge[0:4] - N_ge[1:5]  # count per hi value

        # H[slot] = sum_e L_slot(e) * hi(e)
        H = np.zeros(8)
        Hones = (Nc * np.arange(4)).sum()
        H[4] = Hones
        H[5] = Hones  # s1*hi == hi
        H[6] = Hones - 2 * Nc[1]
        H[7] = Hones - 2 * (Nc[1] + 2 * Nc[2])
        A = np.zeros((4, 4))
        A[0] = 1
        for a in (1, 2, 3):
            A[a] = [1 if h >= a else -1 for h in range(4)]
        D = np.linalg.solve(A, T[0:4])  # per-hi d-marginals
        f = {1: np.array([0., 1, 2, 3]), 2: np.array([0., -1, 2, 3]),
             3: np.array([0., -1, -2, 3])}
        H[0] = (np.arange(4) * D).sum()
        for a in (1, 2, 3):
            H[a] = (f[a] * D).sum()

        Mu = M - H[:, None]  # sum L*[lo>=j] per slot, j=1..7
        Sd = np.concatenate([T[0:4, None], Mu[0:4]], axis=1)
        Sc = np.concatenate([T[4:8, None], Mu[4:8]], axis=1)
        S_ge = np.zeros((5, 8))
        N_ge2 = np.zeros((5, 8))
        S_ge[0] = Sd[0]
        N_ge2[0] = Sc[0]
        for a in (1, 2, 3):
            S_ge[a] = (Sd[a] + Sd[0]) / 2
            N_ge2[a] = (Sc[a] + Sc[0]) / 2
        S_hi = S_ge[0:4] - S_ge[1:5]  # per-hi rows, lo-cumulative cols
        N_hi = N_ge2[0:4] - N_ge2[1:5]

        def lodiff(Mx):
            Mz = np.concatenate([Mx, np.zeros((Mx.shape[0], 1))], axis=1)
            return Mz[:, :8] - Mz[:, 1:9]

        S[c] = lodiff(S_hi).reshape(-1)   # bin = 8*hi + lo
        Cnt[c] = lodiff(N_hi).reshape(-1)
    return S, Cnt


def _finalize(S, Cnt):
    diff = np.where(Cnt > 0, np.abs(S) / np.maximum(Cnt, 1.0), 0.0)
    return np.float32(diff.mean())


def kernel(pred, target, input_img):
    from concourse.bass_utils import run_bass_kernel_spmd

    nc = _get_nc()
    in_maps = _make_in_maps(pred, target, input_img)
    res = run_bass_kernel_spmd(nc, in_maps, list(range(N_CORES)))
    S = np.zeros((C, NB), np.float64)
    Cnt = np.zeros((C, NB), np.float64)
    for r in res.results:
        s, cc = _decode(r["out"])
        S += s
        Cnt += cc
    _CACHE["last_SC"] = (S, Cnt)
    return np.asarray(_finalize(S, Cnt), dtype=np.float32)
